# revision 18
# baseline (speedup 1.0000x reference)
"""Trainium2 Bass kernel for the paired-view ("flip") multi-head attention module.

Full computation (reference semantics, B=2 P=2 S=1024 D=1024 H=16):
    q/k/v = Linear(x) -> [B,P,H,S,DK]
    left  = softmax(q k^T / 8 + mask) v           (same pair index)
    right = softmax(q k_flip^T / 8 + mask) v_flip (pair index swapped)
    out   = (left + 0.1*tanh(right)) @ Wo.T + bo

Sharding over 8 NeuronCores: data-parallel on B (2 groups of 4 cores),
tensor-parallel on heads within a group (4 heads/core, 256 channels).
Each core computes its heads' projections (column-parallel), full attention
for its heads over both pair views, and a row-parallel partial of the output
projection.  The host sums the 4 bf16 partials per batch and adds bo.

Scores are computed TRANSPOSED ([k, q]) so softmax's exp is orientation-free
and the attention-value product needs no on-chip transposes; row sums come
free via an extra ones-column in V (mask is folded into V so exp needs no
bias).  softmax exp is split across ScalarE (native Exp) and a custom DVE
op EXPQ4 computing exp(s/8) = (((a s + b) s + c) s + 1)^4, keeping the
tensor engine fed: every slot interleaves combo j's score matmuls with
combo j-1's AV matmuls so PE never idles (idle windows trip the HAM clock
gate and halve the PE clock for ~3.4us).
"""

import numpy as np

import concourse.bass as bass
import concourse.tile as tile
from concourse import bacc, mybir
from concourse.bass_utils import run_bass_kernel_spmd

F32 = mybir.dt.float32
BF16 = mybir.dt.bfloat16
I32 = mybir.dt.int32

X_DT = BF16    # projection inputs: xT staging + Wq/Wk/Wv
QK_DT = BF16   # q/k tiles feeding the scores matmul
AV_DT = BF16   # exp(scores) + v_aug feeding the AV matmul
OUT_DT = BF16  # combine + Wo feeding the output projection
AF = mybir.ActivationFunctionType
OP = mybir.AluOpType

B, P, S, D, H = 2, 2, 1024, 1024, 16
DK = D // H          # 64
NCORES = 8
GROUP = 4            # cores per batch entry
NH = H // GROUP      # 4 local heads per core
CH = NH * DK         # 256 local channels
R = P * S            # 2048 rows per batch entry
KC = 8               # d_model chunks of 128
VW = NH * 65         # v_aug row-chunk width (4 heads x (64+ones))

# exp(0.125*s) ~ (((A3*s + A2)*s + A1)*s + 1)^4, |s| <= 22 (6.7 sigma)
EXP_A3, EXP_A2, EXP_A1 = 4.948145866546519e-06, 0.0005008251844718205, 0.031308304498639195
DVE_KCS = (1, 4, 6)  # score chunks whose exp runs on DVE (rest on ScalarE)


def _register_expq4():
    from concourse import dve_ops
    from concourse.dve_spec import Spec, Src0, C0, C1, C2, One, lower, sq
    from concourse.dve_uop import DveOpSpec

    for op in dve_ops.OPS:
        if op.name == "EXPQ4":
            return op
    body = sq(sq(((C0 * Src0 + C1) * Src0 + C2) * Src0 + One))
    spec = Spec(
        body=body,
        reference=lambda in0, s0, s1, imm2: (((s0 * in0 + s1) * in0 + imm2) * in0 + 1.0) ** 4,
    )
    opcode = dve_ops._CUSTOM_DVE_ROW_BASE + len(dve_ops.OPS)
    shas = {}
    for ver in ("v3", "v4"):
        d = DveOpSpec(name="EXPQ4", opcode=opcode, uops=lower(spec, ver=ver), rd1_en=False)
        shas[ver] = d.sha(ver)
    op = dve_ops.DveOp("EXPQ4", spec, subdim=False, uops_sha=shas)
    dve_ops.OPS.append(op)
    dve_ops._SUB_OPCODE_FOR_NAME[op.name] = opcode
    dve_ops.CUSTOM_DVE_SPECS[op.name] = op.spec
    return op


EXPQ4 = _register_expq4()


def _emit(nc, tc, xq, xk, xv, wq, wk, wv, wo, bq, bk, bv, mask, out_d):
    from contextlib import ExitStack

    with ExitStack() as ctx:
        sb = ctx.enter_context(tc.tile_pool(name="sb", bufs=1))
        ps = ctx.enter_context(tc.tile_pool(name="ps", bufs=1, space="PSUM"))
        _body(nc, sb, ps, xq, xk, xv, wq, wk, wv, wo, bq, bk, bv, mask, out_d)


def _body(nc, sb, ps, xq, xk, xv, wq, wk, wv, wo, bq, bk, bv, mask, out_d):
    # ---- staging prefetch (4 queues; k0/q0 gate the lead-in) -----------
    _stage_cache = {}
    _stage_engs = [nc.sync, nc.gpsimd, nc.scalar, nc.sync]
    _stage_rr = [0]

    def stage_fetch(kind, rb):
        # 4 quarter-DMAs per chunk spread over the 3 DMA-capable engines so
        # the first proj matmuls aren't gated on one slow 1MB transfer
        if (kind, rb) in _stage_cache:
            return _stage_cache[(kind, rb)]
        src_d = {"q": xq, "k": xk, "v": xv}[kind]
        stage = sb.tile([128, KC * 512], X_DT, name="stage", tag="stage", bufs=4)
        _stage_cache[(kind, rb)] = stage
        for quar in range(4):
            eng = _stage_engs[_stage_rr[0] % len(_stage_engs)]
            _stage_rr[0] += 1
            eng.dma_start(
                out=stage[:, quar * 1024 : (quar + 1) * 1024].rearrange(
                    "p (kc c) -> p kc c", kc=2
                ),
                in_=src_d[
                    quar * 256 : (quar + 1) * 256,
                    rb * 512 : (rb + 1) * 512,
                ].rearrange("(kc p) c -> p kc c", p=128),
            )
        return stage

    # ---- constants (wk/wq split in halves, gating the lead-in) ---------
    wq_sb = sb.tile([128, KC * CH], X_DT, name="wq_sb")
    wk_sb = sb.tile([128, KC * CH], X_DT, name="wk_sb")
    wv_sb = sb.tile([128, KC * CH], X_DT, name="wv_sb")
    for i, (t_d, t_s) in enumerate(((wk, wk_sb), (wq, wq_sb))):
        for half in range(2):
            _stage_engs[(2 * i + half) % 3].dma_start(
                out=t_s[:, half * 4 * CH : (half + 1) * 4 * CH].rearrange(
                    "p (kc c) -> p kc c", kc=KC // 2
                ),
                in_=t_d[half * 512 : (half + 1) * 512, :].rearrange(
                    "(kc p) c -> p kc c", p=128
                ),
            )
    stage_fetch("k", 0)
    stage_fetch("q", 0)
    stage_fetch("k", 1)
    stage_fetch("q", 1)
    nc.gpsimd.dma_start(
        out=wv_sb[:].rearrange("p (kc c) -> p kc c", kc=KC),
        in_=wv[:].rearrange("(kc p) c -> p kc c", p=128),
    )
    wo_sb = sb.tile([128, 2 * D], OUT_DT, name="wo_sb")
    nc.scalar.dma_start(
        out=wo_sb[:].rearrange("p (kk c) -> p kk c", kk=2),
        in_=wo[:].rearrange("(kk p) c -> p kk c", p=128),
    )

    bq_sb = sb.tile([128, 2], F32, name="bq_sb")
    bk_sb = sb.tile([128, 2], F32, name="bk_sb")
    nc.sync.dma_start(out=bq_sb[:], in_=bq[:].rearrange("(mo p) -> p mo", p=128))
    nc.sync.dma_start(out=bk_sb[:], in_=bk[:].rearrange("(mo p) -> p mo", p=128))
    bv_row = sb.tile([1, CH], F32, name="bv_row")
    nc.sync.dma_start(out=bv_row[:], in_=bv[None, :])
    bv_bc = sb.tile([128, CH], F32, name="bv_bc")
    nc.gpsimd.partition_broadcast(bv_bc[:], bv_row[:])

    # mask as a 0/1 multiplier on v_aug rows (kills masked keys in both the
    # attention numerator and the ones-column denominator)
    mask_sb = sb.tile([128, 2 * KC], I32, name="mask_sb")
    nc.sync.dma_start(
        out=mask_sb[:],
        in_=mask[:].rearrange("pp (kc p) -> p pp kc", p=128),
    )
    mask_f = sb.tile([128, 2 * KC], F32, name="mask_f")
    nc.gpsimd.tensor_copy(mask_f[:], mask_sb[:])
    _mask_ones_done = []

    # ---- projections --------------------------------------------------
    qT = [sb.tile([128, R], QK_DT, name=f"qT{mo}") for mo in range(2)]
    kT = [sb.tile([128, R], QK_DT, name=f"kT{mo}") for mo in range(2)]
    # v_aug: [r_local, rc(16) x (h(4) x 65)]; col h*65+64 holds ones (masked)
    v_aug = sb.tile([128, 16 * VW], AV_DT, name="v_aug")
    nc.gpsimd.memset(v_aug[:], 1.0)
    # ones columns carry the key mask (denominator masking)
    for h in range(NH):
        nc.vector.tensor_copy(
            v_aug[:].rearrange("p (rc w) -> p rc w", rc=16)[
                :, :, h * 65 + 64 : h * 65 + 65
            ],
            mask_f[:, :, None],
        )

    def proj_chunk(kind, rb):
        stage = stage_fetch(kind, rb)
        w_sb = {"q": wq_sb, "k": wk_sb, "v": wv_sb}[kind]
        if kind in ("q", "k"):
            dst, b_sb = (qT, bq_sb) if kind == "q" else (kT, bk_sb)
            for mo in range(2):
                pp_t = ps.tile([128, 512], F32, name="ps_w", tag="ps_w", bufs=2)
                for kc in range(KC):
                    nc.tensor.matmul(
                        pp_t[:],
                        w_sb[:, kc * CH + mo * 128 : kc * CH + (mo + 1) * 128],
                        stage[:, kc * 512 : (kc + 1) * 512],
                        start=(kc == 0),
                        stop=(kc == KC - 1),
                    )
                nc.vector.tensor_scalar(
                    out=dst[mo][:, rb * 512 : (rb + 1) * 512],
                    in0=pp_t[:],
                    scalar1=b_sb[:, mo : mo + 1],
                    scalar2=None,
                    op0=OP.add,
                )
        else:
            for rs in range(4):
                rc = rb * 4 + rs
                pv_t = ps.tile([128, CH], F32, name="ps_v", tag="ps_w", bufs=2)
                for kc in range(KC):
                    nc.tensor.matmul(
                        pv_t[:],
                        stage[:, kc * 512 + rs * 128 : kc * 512 + (rs + 1) * 128],
                        wv_sb[:, kc * CH : (kc + 1) * CH],
                        start=(kc == 0),
                        stop=(kc == KC - 1),
                    )
                # (pv * mask) + bv: exact key masking for bv == 0 (the ones
                # column carries the mask so the denominator is always right)
                blk = v_aug[:, rc * VW : (rc + 1) * VW]
                nc.vector.scalar_tensor_tensor(
                    out=blk.rearrange("p (h x) -> p h x", h=NH)[:, :, 0:DK],
                    in0=pv_t[:].rearrange("p (h x) -> p h x", h=NH),
                    scalar=mask_f[:, rc : rc + 1],
                    in1=bv_bc[:].rearrange("p (h x) -> p h x", h=NH),
                    op0=OP.mult,
                    op1=OP.add,
                )

    # ---- attention ----------------------------------------------------
    comb = [sb.tile([128, R], OUT_DT, name=f"comb{kk}") for kk in range(2)]
    avs = {}

    def new_ex():
        return [
            sb.tile([128, 4096], AV_DT, name="ex", tag="ex", bufs=4) for _ in range(2)
        ]

    def emit_exp(ex, ss_ap, kc, qb=None):
        lo = (kc % 4) * 1024 + (0 if qb is None else qb * 512)
        hi = lo + (1024 if qb is None else 512)
        dst = ex[kc // 4][:, lo:hi]
        if kc in DVE_KCS:
            nc.vector._custom_dve(
                EXPQ4, out=dst, in0=ss_ap, s0=EXP_A3, s1=EXP_A2, imm2=EXP_A1
            )
        else:
            nc.scalar.activation(dst, ss_ap, AF.Exp, scale=0.125)

    def score_kc(combo, ex, kc):
        p, h, side = combo
        mo, po = h // 2, (h % 2) * 64
        pp = p if side == 0 else 1 - p
        ss = ps.tile([128, 1024], F32, name="ps_s", tag="ps_s", bufs=2)
        for qb in range(2):
            nc.tensor.matmul(
                ss[:, qb * 512 : (qb + 1) * 512],
                kT[mo][po : po + 64, pp * S + kc * 128 : pp * S + (kc + 1) * 128],
                qT[mo][po : po + 64, p * S + qb * 512 : p * S + (qb + 1) * 512],
                start=True,
                stop=True,
            )
        emit_exp(ex, ss[:], kc)

    def av_group(combo, ex, st, g):
        # g in 0..3: qb = g//2, kc span (g%2)*4..+4; st holds [pa_t per qb]
        p, h, side = combo
        pp = p if side == 0 else 1 - p
        qb, sub = g // 2, g % 2
        if sub == 0:
            st[qb] = ps.tile([65, 512], F32, name="ps_av", tag="ps_av", bufs=2)
        pa = st[qb]
        for k2 in range(4):
            kc = sub * 4 + k2
            nc.tensor.matmul(
                pa[:],
                v_aug[
                    :, (pp * KC + kc) * VW + h * 65 : (pp * KC + kc) * VW + (h + 1) * 65
                ],
                ex[kc // 4][:, (kc % 4) * 1024 + qb * 512 : (kc % 4) * 1024 + (qb + 1) * 512],
                start=(sub == 0 and k2 == 0),
                stop=(sub == 1 and k2 == 3),
            )
        if sub == 1:
            av = avs[combo]
            nc.scalar.copy(av[:, qb * 512 : (qb + 1) * 512], pa[:])

    _pair = {}
    _pre_state = {}

    def combine_pre(p, h):
        # reciprocal of the two sums rows + broadcast; no consumer blocks on
        # this chain until combine_fin one slot later
        avL, avR = avs.pop((p, h, 0)), avs.pop((p, h, 1))
        srs = sb.tile([32, 64], F32, name="srs", tag="srs", bufs=2)
        nc.sync.dma_start(
            out=srs[0:16, :], in_=avL[64:65, :].rearrange("p (m e) -> p m e", e=64)
        )
        nc.sync.dma_start(
            out=srs[16:32, :], in_=avR[64:65, :].rearrange("p (m e) -> p m e", e=64)
        )
        rrs = sb.tile([32, 64], F32, name="rrs", tag="rrs", bufs=2)
        nc.vector.reciprocal(rrs[:], srs[:])
        rr2 = sb.tile([1, 2 * S], F32, name="rr2", tag="rrow", bufs=2)
        nc.sync.dma_start(
            out=rr2[:, 0:S].rearrange("p (m e) -> p m e", e=64), in_=rrs[0:16, :]
        )
        nc.sync.dma_start(
            out=rr2[:, S : 2 * S].rearrange("p (m e) -> p m e", e=64), in_=rrs[16:32, :]
        )
        bc2 = sb.tile([64, 2 * S], F32, name="bc2", tag="bc", bufs=2)
        nc.gpsimd.partition_broadcast(bc2[:], rr2[:])
        _pre_state[(p, h)] = (avL, avR, bc2)

    def combine_fin(p, h):
        avL, avR, bc2 = _pre_state.pop((p, h))
        po = (h % 2) * 64
        if h % 2 == 0:
            t1p = sb.tile([128, S], F32, name="t1p", tag="t1", bufs=2)
            t2p = sb.tile([128, S], F32, name="t2p", tag="t2", bufs=2)
            _pair[(p, h // 2)] = (t1p, t2p)
        else:
            t1p, t2p = _pair[(p, h // 2)]
        nc.gpsimd.tensor_tensor(
            out=t1p[po : po + 64, :], in0=avL[0:64, :], in1=bc2[:, 0:S], op=OP.mult
        )
        nc.vector.tensor_tensor(
            out=t2p[po : po + 64, :], in0=avR[0:64, :], in1=bc2[:, S : 2 * S], op=OP.mult
        )
        if h % 2 == 1:
            t3p = sb.tile([128, S], F32, name="t3p", tag="t3", bufs=1)
            nc.scalar.activation(t3p[:], t2p[:], AF.Tanh)
            nc.vector.scalar_tensor_tensor(
                out=comb[h // 2][:, p * S : (p + 1) * S],
                in0=t3p[:],
                scalar=0.1,
                in1=t1p[:],
                op0=OP.mult,
                op1=OP.add,
            )

    _op_n = [0]

    def outproj_rc(p, rc):
        od = sb.tile([128, D], OUT_DT, name="od", tag="od", bufs=3)
        for ob in range(2):
            po_t = ps.tile([128, 512], F32, name="ps_o", tag="ps_w", bufs=2)
            for kk in range(2):
                nc.tensor.matmul(
                    po_t[:],
                    comb[kk][:, p * S + rc * 128 : p * S + (rc + 1) * 128],
                    wo_sb[:, kk * D + ob * 512 : kk * D + (ob + 1) * 512],
                    start=(kk == 0),
                    stop=(kk == 1),
                )
            nc.vector.tensor_copy(od[:, ob * 512 : (ob + 1) * 512], po_t[:])
            _op_n[0] += 1
        (nc.sync if rc % 2 == 0 else nc.gpsimd).dma_start(
            out=out_d[p * S + rc * 128 : p * S + (rc + 1) * 128, :], in_=od[:]
        )

    # ---- schedule -----------------------------------------------------
    # combos per p: L0..L3 then R0..R3; slot j = scores(j) + AV(j-1) with
    # AV interleaved at kc-pair grain; eager per-h combines; proj chunks and
    # the other p's output projection fill the PE between slots.
    combos = [(p, h, side) for p in range(2) for side in range(2) for h in range(NH)]

    # fillers keyed by combo index (run at slot start, before scores)
    fill = {
        1: [("proj", "v", 0), ("proj", "v", 1)],
        2: [("proj", "k", 2)],
        3: [("proj", "k", 3)],
        4: [("proj", "v", 2)],
        5: [("proj", "v", 3)],
        6: [("proj", "q", 2)],
        7: [("proj", "q", 3)],
        10: [("out", 0, 0), ("out", 0, 1)],
        11: [("out", 0, 2), ("out", 0, 3)],
        12: [("out", 0, 4)],
        13: [("out", 0, 5)],
        14: [("out", 0, 6)],
        15: [("out", 0, 7)],
    }

    def run_filler(f):
        if f[0] == "proj":
            proj_chunk(f[1], f[2])
        else:
            outproj_rc(f[1], f[2])

    # lead-in: k0,q0 -> L0 scores qb0 halves; k1 mid; q1 -> qb1 halves
    proj_chunk("k", 0)
    proj_chunk("q", 0)
    ex0 = new_ex()
    combo0 = combos[0]
    avs[combo0] = sb.tile([65, S], F32, name="av", tag="avT", bufs=7)
    for kc in range(KC):
        if kc == 4:
            proj_chunk("k", 1)
        ss = ps.tile([128, 1024], F32, name="ps_s", tag="ps_s", bufs=2)
        nc.tensor.matmul(
            ss[:, 0:512],
            kT[0][0:64, kc * 128 : (kc + 1) * 128],
            qT[0][0:64, 0:512],
            start=True,
            stop=True,
        )
        emit_exp(ex0, ss[:, 0:512], kc, qb=0)
    proj_chunk("q", 1)
    for kc in range(KC):
        ss = ps.tile([128, 1024], F32, name="ps_s", tag="ps_s", bufs=2)
        nc.tensor.matmul(
            ss[:, 0:512],
            kT[0][0:64, kc * 128 : (kc + 1) * 128],
            qT[0][0:64, 512:1024],
            start=True,
            stop=True,
        )
        emit_exp(ex0, ss[:, 0:512], kc, qb=1)

    prev = (combo0, ex0)
    pend_pre, pend_fin = [], []
    for j in range(1, len(combos) + 1):
        combo = combos[j] if j < len(combos) else None
        for f in fill.get(j, []):
            run_filler(f)
        if combo is not None:
            avs[combo] = sb.tile([65, S], F32, name="av", tag="avT", bufs=7)
            ex = new_ex()
        st = [None, None]
        for kc in range(KC):
            if combo is not None:
                score_kc(combo, ex, kc)
            if prev is not None and kc % 2 == 1:
                av_group(prev[0], prev[1], st, kc // 2)
        if prev is not None and prev[0][2] == 1:  # R side AV done
            pend_pre.append((prev[0][0], prev[0][1]))
        if pend_fin:
            combine_fin(*pend_fin.pop(0))
        if pend_pre:
            ph = pend_pre.pop(0)
            combine_pre(*ph)
            pend_fin.append(ph)
        prev = (combo, ex) if combo is not None else None

    # tail: last combine + p1 output projection
    while pend_fin:
        combine_fin(*pend_fin.pop(0))
    for rc in range(8):
        outproj_rc(1, rc)


_CACHED = None


def _build():
    global _CACHED
    if _CACHED is not None:
        return _CACHED
    nc = bacc.Bacc("TRN2", target_bir_lowering=False, debug=False)
    xq = nc.dram_tensor("xq", [D, R], X_DT, kind="ExternalInput")
    xk = nc.dram_tensor("xk", [D, R], X_DT, kind="ExternalInput")
    xv = nc.dram_tensor("xv", [D, R], X_DT, kind="ExternalInput")
    wq = nc.dram_tensor("wq", [D, CH], X_DT, kind="ExternalInput")
    wk = nc.dram_tensor("wk", [D, CH], X_DT, kind="ExternalInput")
    wv = nc.dram_tensor("wv", [D, CH], X_DT, kind="ExternalInput")
    wo = nc.dram_tensor("wo", [CH, D], OUT_DT, kind="ExternalInput")
    bq = nc.dram_tensor("bq", [CH], F32, kind="ExternalInput")
    bk = nc.dram_tensor("bk", [CH], F32, kind="ExternalInput")
    bv = nc.dram_tensor("bv", [CH], F32, kind="ExternalInput")
    mask = nc.dram_tensor("mask", [P, S], I32, kind="ExternalInput")
    out_d = nc.dram_tensor("out", [R, D], OUT_DT, kind="ExternalOutput")
    with tile.TileContext(nc) as tc:
        _emit(nc, tc, xq, xk, xv, wq, wk, wv, wo, bq, bk, bv, mask, out_d)
    nc.compile()
    _CACHED = nc
    return nc


def _in_maps(query, key, value, mask, Wq, bq, Wk, bk, Wv, bv, Wo):
    xnp = mybir.dt.np(X_DT)
    onp = mybir.dt.np(OUT_DT)
    f32 = lambda a: np.ascontiguousarray(np.asarray(a, dtype=np.float32))
    xdt = lambda a: np.ascontiguousarray(np.asarray(a).astype(xnp))
    odt = lambda a: np.ascontiguousarray(np.asarray(a).astype(onp))
    query, key, value = f32(query), f32(key), f32(value)
    Wq, Wk, Wv, Wo = f32(Wq), f32(Wk), f32(Wv), f32(Wo)
    bq, bk, bv = f32(bq), f32(bk), f32(bv)
    mask = np.ascontiguousarray(np.asarray(mask, dtype=np.int32))

    xqT = [xdt(query[b].reshape(R, D).T) for b in range(B)]
    xkT = [xdt(key[b].reshape(R, D).T) for b in range(B)]
    xvT = [xdt(value[b].reshape(R, D).T) for b in range(B)]

    maps = []
    for c in range(NCORES):
        b, hg = divmod(c, GROUP)
        ch = slice(hg * CH, (hg + 1) * CH)
        maps.append(
            {
                "xq": xqT[b],
                "xk": xkT[b],
                "xv": xvT[b],
                "wq": xdt(Wq[ch, :].T),
                "wk": xdt(Wk[ch, :].T),
                "wv": xdt(Wv[ch, :].T),
                "wo": odt(Wo[:, ch].T),
                "bq": bq[ch],
                "bk": bk[ch],
                "bv": bv[ch],
                "mask": mask[b, :, 0, :],
            }
        )
    return maps


def _run(in_maps, **kwargs):
    nc = _build()
    return run_bass_kernel_spmd(nc, in_maps, core_ids=list(range(NCORES)), **kwargs)


def kernel(query, key, value, mask, Wq, bq, Wk, bk, Wv, bv, Wo, bo):
    res = _run(_in_maps(query, key, value, mask, Wq, bq, Wk, bk, Wv, bv, Wo))
    bo = np.asarray(bo, dtype=np.float32)
    out = np.zeros((B, P, S, D), dtype=np.float32)
    for c in range(NCORES):
        b = c // GROUP
        out[b] += res.results[c]["out"].astype(np.float32).reshape(P, S, D)
    out += bo
    return out


# revision 32
# speedup vs baseline: 1.0916x; 1.0916x over previous
"""Trainium2 Bass kernel for the paired-view ("flip") multi-head attention module.

Full computation (reference semantics, B=2 P=2 S=1024 D=1024 H=16):
    q/k/v = Linear(x) -> [B,P,H,S,DK]
    left  = softmax(q k^T / 8 + mask) v           (same pair index)
    right = softmax(q k_flip^T / 8 + mask) v_flip (pair index swapped)
    out   = (left + 0.1*tanh(right)) @ Wo.T + bo

Sharding over 8 NeuronCores: data-parallel on B (2 groups of 4 cores),
tensor-parallel on heads within a group (4 heads/core, 256 channels).
Each core computes its heads' projections (column-parallel), full attention
for its heads over both pair views, and a row-parallel partial of the output
projection.  The host sums the 4 bf16 partials per batch and adds bo.

Scores are computed TRANSPOSED ([k, q]) so softmax's exp is orientation-free
and the attention-value product needs no on-chip transposes; row sums come
free via an extra ones-column in V (mask is folded into V so exp needs no
bias).  softmax exp is split across ScalarE (native Exp) and a custom DVE
op EXPQ4 computing exp(s/8) = (((a s + b) s + c) s + 1)^4, keeping the
tensor engine fed: every slot interleaves combo j's score matmuls with
combo j-1's AV matmuls so PE never idles (idle windows trip the HAM clock
gate and halve the PE clock for ~3.4us).
"""

import numpy as np

import concourse.bass as bass
import concourse.tile as tile
from concourse import bacc, mybir
from concourse.bass_utils import run_bass_kernel_spmd

F32 = mybir.dt.float32
F32R = mybir.dt.float32r
BF16 = mybir.dt.bfloat16
I32 = mybir.dt.int32

X_DT = BF16    # projection inputs: xT staging + Wq/Wk/Wv
QK_DT = BF16   # q/k tiles feeding the scores matmul
AV_DT = BF16   # exp(scores) + v_aug feeding the AV matmul
OUT_DT = BF16  # combine + Wo feeding the output projection
AF = mybir.ActivationFunctionType
OP = mybir.AluOpType

B, P, S, D, H = 2, 2, 1024, 1024, 16
DK = D // H          # 64
NCORES = 8
GROUP = 4            # cores per batch entry
NH = H // GROUP      # 4 local heads per core
CH = NH * DK         # 256 local channels
R = P * S            # 2048 rows per batch entry
KC = 8               # d_model chunks of 128
VW = NH * 128        # v_aug row-chunk width (4 heads x [ones|pad|64 values])

# exp(0.125*s) ~ (((A3*s + A2)*s + A1)*s + 1)^4, |s| <= 22 (6.7 sigma)
EXP_A3, EXP_A2, EXP_A1 = 4.948145866546519e-06, 0.0005008251844718205, 0.031308304498639195
DVE_KCS = (1, 4, 6)  # score chunks whose exp runs on DVE (rest on ScalarE)


def _register_expq4():
    from concourse import dve_ops
    from concourse.dve_spec import Spec, Src0, C0, C1, C2, One, lower, sq
    from concourse.dve_uop import DveOpSpec

    for op in dve_ops.OPS:
        if op.name == "EXPQ4":
            return op
    body = sq(sq(((C0 * Src0 + C1) * Src0 + C2) * Src0 + One))
    spec = Spec(
        body=body,
        reference=lambda in0, s0, s1, imm2: (((s0 * in0 + s1) * in0 + imm2) * in0 + 1.0) ** 4,
    )
    opcode = dve_ops._CUSTOM_DVE_ROW_BASE + len(dve_ops.OPS)
    shas = {}
    for ver in ("v3", "v4"):
        d = DveOpSpec(name="EXPQ4", opcode=opcode, uops=lower(spec, ver=ver), rd1_en=False)
        shas[ver] = d.sha(ver)
    op = dve_ops.DveOp("EXPQ4", spec, subdim=False, uops_sha=shas)
    dve_ops.OPS.append(op)
    dve_ops._SUB_OPCODE_FOR_NAME[op.name] = opcode
    dve_ops.CUSTOM_DVE_SPECS[op.name] = op.spec
    return op


EXPQ4 = _register_expq4()


def _emit(nc, tc, xq, xk, xv, wq, wk, wv, wo, bq, bk, bv, mask, out_d):
    from contextlib import ExitStack

    with ExitStack() as ctx:
        sb = ctx.enter_context(tc.tile_pool(name="sb", bufs=1))
        ps = ctx.enter_context(tc.tile_pool(name="ps", bufs=1, space="PSUM"))
        _body(nc, sb, ps, xq, xk, xv, wq, wk, wv, wo, bq, bk, bv, mask, out_d)


def _body(nc, sb, ps, xq, xk, xv, wq, wk, wv, wo, bq, bk, bv, mask, out_d):
    # ---- staging prefetch (4 queues; k0/q0 gate the lead-in) -----------
    _stage_cache = {}
    _stage_engs = [nc.sync, nc.gpsimd, nc.scalar, nc.sync]
    _stage_rr = [0]

    def stage_fetch(kind, rb, engs=None):
        # 4 quarter-DMAs per chunk spread over the 3 DMA-capable engines so
        # the first proj matmuls aren't gated on one slow 1MB transfer
        if (kind, rb) in _stage_cache:
            return _stage_cache[(kind, rb)]
        src_d = {"q": xq, "k": xk, "v": xv}[kind]
        stage = sb.tile([128, KC * 512], X_DT, name="stage", tag="stage", bufs=3)
        _stage_cache[(kind, rb)] = stage
        for quar in range(4):
            if engs is not None:
                eng = engs[quar]
            else:
                eng = _stage_engs[_stage_rr[0] % len(_stage_engs)]
                _stage_rr[0] += 1
            eng.dma_start(
                out=stage[:, quar * 1024 : (quar + 1) * 1024].rearrange(
                    "p (kc c) -> p kc c", kc=2
                ),
                in_=src_d[
                    quar * 256 : (quar + 1) * 256,
                    rb * 512 : (rb + 1) * 512,
                ].rearrange("(kc p) c -> p kc c", p=128),
            )
        return stage

    # ---- constants; DMA issue order = queue order, so k-side first -----
    wq_sb = sb.tile([128, KC * CH], X_DT, name="wq_sb")
    wk_sb = sb.tile([128, KC * CH], X_DT, name="wk_sb")
    wv_sb = sb.tile([128, KC * CH], X_DT, name="wv_sb")

    def w_load(t_d, t_s, engs):
        for half in range(2):
            engs[half].dma_start(
                out=t_s[:, half * 4 * CH : (half + 1) * 4 * CH].rearrange(
                    "p (kc c) -> p kc c", kc=KC // 2
                ),
                in_=t_d[half * 512 : (half + 1) * 512, :].rearrange(
                    "(kc p) c -> p kc c", p=128
                ),
            )

    # wave 1: everything proj("k",0) needs
    w_load(wk, wk_sb, (nc.sync, nc.gpsimd))
    stage_fetch("k", 0, engs=(nc.scalar, nc.sync, nc.gpsimd, nc.scalar))
    # wave 2: proj("q",0)
    w_load(wq, wq_sb, (nc.sync, nc.gpsimd))
    stage_fetch("q", 0, engs=(nc.scalar, nc.sync, nc.gpsimd, nc.scalar))
    # wave 3: the rest of the lead
    stage_fetch("k", 1, engs=(nc.sync, nc.gpsimd, nc.scalar, nc.sync))
    stage_fetch("q", 1, engs=(nc.gpsimd, nc.scalar, nc.sync, nc.gpsimd))
    nc.gpsimd.dma_start(
        out=wv_sb[:].rearrange("p (kc c) -> p kc c", kc=KC),
        in_=wv[:].rearrange("(kc p) c -> p kc c", p=128),
    )
    wo_sb = sb.tile([128, 2 * D], OUT_DT, name="wo_sb")
    nc.scalar.dma_start(
        out=wo_sb[:].rearrange("p (kk c) -> p kk c", kk=2),
        in_=wo[:].rearrange("(kk p) c -> p kk c", p=128),
    )

    bq_sb = sb.tile([128, 2], F32, name="bq_sb")
    bk_sb = sb.tile([128, 2], F32, name="bk_sb")
    nc.sync.dma_start(out=bq_sb[:], in_=bq[:].rearrange("(mo p) -> p mo", p=128))
    nc.sync.dma_start(out=bk_sb[:], in_=bk[:].rearrange("(mo p) -> p mo", p=128))
    bv_row = sb.tile([1, CH], F32, name="bv_row")
    nc.sync.dma_start(out=bv_row[:], in_=bv[None, :])
    bv_bc = sb.tile([128, CH], F32, name="bv_bc")
    nc.gpsimd.partition_broadcast(bv_bc[:], bv_row[:])

    # mask as a 0/1 multiplier on v_aug rows (kills masked keys in both the
    # attention numerator and the ones-column denominator)
    mask_sb = sb.tile([128, 2 * KC], I32, name="mask_sb")
    nc.sync.dma_start(
        out=mask_sb[:],
        in_=mask[:].rearrange("pp (kc p) -> p pp kc", p=128),
    )
    mask_f = sb.tile([128, 2 * KC], F32, name="mask_f")
    nc.gpsimd.tensor_copy(mask_f[:], mask_sb[:])
    _mask_ones_done = []

    # ---- projections --------------------------------------------------
    qT = [sb.tile([128, R], QK_DT, name=f"qT{mo}") for mo in range(2)]
    kT = [sb.tile([128, R], QK_DT, name=f"kT{mo}") for mo in range(2)]
    # v_aug: [r_local, rc(16) x (h(4) x 65)]; col h*65+64 holds ones (masked)
    v_aug = sb.tile([128, 16 * VW], AV_DT, name="v_aug")
    nc.gpsimd.memset(v_aug[:], 1.0)
    ones_t = sb.tile([1, 64], F32, name="ones_t")
    nc.vector.memset(ones_t[:], 1.0)
    # ones columns carry the key mask (denominator masking)
    for h in range(NH):
        nc.vector.tensor_copy(
            v_aug[:].rearrange("p (rc w) -> p rc w", rc=16)[
                :, :, h * 128 : h * 128 + 1
            ],
            mask_f[:, :, None],
        )

    def proj_chunk(kind, rb):
        stage = stage_fetch(kind, rb)
        w_sb = {"q": wq_sb, "k": wk_sb, "v": wv_sb}[kind]
        if kind in ("q", "k"):
            dst, b_sb = (qT, bq_sb) if kind == "q" else (kT, bk_sb)
            for mo in range(2):
                pp_t = ps.tile([128, 512], F32, name="ps_w", tag="ps_w", bufs=2)
                for kc in range(KC):
                    nc.tensor.matmul(
                        pp_t[:],
                        w_sb[:, kc * CH + mo * 128 : kc * CH + (mo + 1) * 128],
                        stage[:, kc * 512 : (kc + 1) * 512],
                        start=(kc == 0),
                        stop=(kc == KC - 1),
                    )
                nc.vector.tensor_scalar(
                    out=dst[mo][:, rb * 512 : (rb + 1) * 512],
                    in0=pp_t[:],
                    scalar1=b_sb[:, mo : mo + 1],
                    scalar2=None,
                    op0=OP.add,
                )
        else:
            for rs in range(4):
                rc = rb * 4 + rs
                pv_t = ps.tile([128, CH], F32, name="ps_v", tag="ps_w", bufs=2)
                for kc in range(KC):
                    nc.tensor.matmul(
                        pv_t[:],
                        stage[:, kc * 512 + rs * 128 : kc * 512 + (rs + 1) * 128],
                        wv_sb[:, kc * CH : (kc + 1) * CH],
                        start=(kc == 0),
                        stop=(kc == KC - 1),
                    )
                # (pv * mask) + bv: exact key masking for bv == 0 (the ones
                # column carries the mask so the denominator is always right)
                blk = v_aug[:, rc * VW : (rc + 1) * VW]
                nc.vector.scalar_tensor_tensor(
                    out=blk.rearrange("p (h x) -> p h x", h=NH)[:, :, 64:128],
                    in0=pv_t[:].rearrange("p (h x) -> p h x", h=NH),
                    scalar=mask_f[:, rc : rc + 1],
                    in1=bv_bc[:].rearrange("p (h x) -> p h x", h=NH),
                    op0=OP.mult,
                    op1=OP.add,
                )

    # ---- attention ----------------------------------------------------
    comb = [sb.tile([128, R], OUT_DT, name=f"comb{kk}") for kk in range(2)]
    avs = {}

    def new_ex():
        return [
            sb.tile([128, 4096], AV_DT, name="ex", tag="ex", bufs=4) for _ in range(2)
        ]

    def emit_exp(ex, ss_ap, kc, qb=None):
        lo = (kc % 4) * 1024 + (0 if qb is None else qb * 512)
        hi = lo + (1024 if qb is None else 512)
        dst = ex[kc // 4][:, lo:hi]
        if kc in DVE_KCS:
            nc.vector._custom_dve(
                EXPQ4, out=dst, in0=ss_ap, s0=EXP_A3, s1=EXP_A2, imm2=EXP_A1
            )
        else:
            nc.scalar.activation(dst, ss_ap, AF.Exp, scale=0.125)

    def score_kc(combo, ex, kc):
        p, h, side = combo
        mo, po = h // 2, (h % 2) * 64
        pp = p if side == 0 else 1 - p
        ss = ps.tile([128, 1024], F32, name="ps_s", tag="ps_s", bufs=2)
        for qb in range(2):
            nc.tensor.matmul(
                ss[:, qb * 512 : (qb + 1) * 512],
                kT[mo][po : po + 64, pp * S + kc * 128 : pp * S + (kc + 1) * 128],
                qT[mo][po : po + 64, p * S + qb * 512 : p * S + (qb + 1) * 512],
                start=True,
                stop=True,
            )
        emit_exp(ex, ss[:], kc)

    def av_group(combo, ex, st, g):
        # g in 0..3: qb = g//2, kc span (g%2)*4..+4; st holds [pa_t per qb]
        p, h, side = combo
        pp = p if side == 0 else 1 - p
        qb, sub = g // 2, g % 2
        if sub == 0:
            st[qb] = ps.tile([128, 512], F32, name="ps_av", tag="ps_av", bufs=2)
        pa = st[qb]
        for k2 in range(4):
            kc = sub * 4 + k2
            nc.tensor.matmul(
                pa[:],
                v_aug[
                    :, (pp * KC + kc) * VW + h * 128 : (pp * KC + kc) * VW + (h + 1) * 128
                ],
                ex[kc // 4][:, (kc % 4) * 1024 + qb * 512 : (kc % 4) * 1024 + (qb + 1) * 512],
                start=(sub == 0 and k2 == 0),
                stop=(sub == 1 and k2 == 3),
            )
        if sub == 1:
            av = avs[combo]
            if (h + side) % 2 == 0:
                nc.scalar.copy(av[:, qb * 512 : (qb + 1) * 512], pa[:])
            else:
                nc.vector.tensor_copy(av[:, qb * 512 : (qb + 1) * 512], pa[:])

    _pair = {}
    _pre_state = {}

    def combine_pre(p, h, pe_bc=False):
        # reciprocal of the two sums rows + broadcast, all on-engine (no DMA
        # round-trips); no consumer blocks on this until combine_fin a slot
        # later.  pe_bc: broadcast via a rank-1 f32r matmul (tail only, when
        # the PE is otherwise idle and Pool latency would gate the epilogue).
        avL, avR = avs.pop((p, h, 0)), avs.pop((p, h, 1))
        # sums sit on partition 0 (ones-column is FIRST in each head block):
        # custom DVE ops and partition_broadcast only operate base-partition-0
        nc.vector.reciprocal_approx_fast(avL[0:1, :], avL[0:1, :])
        nc.vector.reciprocal_approx_fast(avR[0:1, :], avR[0:1, :])
        if pe_bc:
            bcL = ps.tile([64, S], F32, name="bcA", tag="ps_s", bufs=2)
            bcR = ps.tile([64, S], F32, name="bcB", tag="ps_s", bufs=2)
            for bc, rr in ((bcL, avL), (bcR, avR)):
                for hf in range(2):
                    nc.tensor.matmul(
                        bc[:, hf * 512 : (hf + 1) * 512],
                        ones_t[:].bitcast(F32R),
                        rr[0:1, hf * 512 : (hf + 1) * 512].bitcast(F32R),
                        start=True,
                        stop=True,
                    )
            bcL_ap, bcR_ap = bcL[:], bcR[:]
        else:
            bcL_t = sb.tile([128, S], F32, name="bcL", tag="bcL", bufs=2)
            bcR_t = sb.tile([128, S], F32, name="bcR", tag="bcR", bufs=2)
            nc.gpsimd.partition_broadcast(bcL_t[:], avL[0:1, :])
            nc.gpsimd.partition_broadcast(bcR_t[:], avR[0:1, :])
            bcL_ap, bcR_ap = bcL_t[64:128, :], bcR_t[64:128, :]
        _pre_state[(p, h)] = (avL, avR, bcL_ap, bcR_ap, pe_bc)

    def combine_fin(p, h):
        avL, avR, bcL_ap, bcR_ap, pe_bc = _pre_state.pop((p, h))
        po = (h % 2) * 64
        if h % 2 == 0:
            t1p = sb.tile([128, S], F32, name="t1p", tag="t1", bufs=2)
            t2p = sb.tile([128, S], F32, name="t2p", tag="t2", bufs=2)
            _pair[(p, h // 2)] = (t1p, t2p)
        else:
            t1p, t2p = _pair[(p, h // 2)]
        # Pool cannot read PSUM, so the pe_bc variant multiplies on DVE
        e1 = nc.vector if pe_bc else nc.gpsimd
        e1.tensor_tensor(
            out=t1p[po : po + 64, :], in0=avL[64:128, :], in1=bcL_ap, op=OP.mult
        )
        nc.vector.tensor_tensor(
            out=t2p[po : po + 64, :], in0=avR[64:128, :], in1=bcR_ap, op=OP.mult
        )
        if h % 2 == 1:
            t3p = sb.tile([128, S], F32, name="t3p", tag="t3", bufs=1)
            nc.scalar.activation(t3p[:], t2p[:], AF.Tanh)
            nc.vector.scalar_tensor_tensor(
                out=comb[h // 2][:, p * S : (p + 1) * S],
                in0=t3p[:],
                scalar=0.1,
                in1=t1p[:],
                op0=OP.mult,
                op1=OP.add,
            )

    _op_n = [0]

    def outproj_rc(p, rc):
        od = sb.tile([128, D], OUT_DT, name="od", tag="od", bufs=3)
        for ob in range(2):
            po_t = ps.tile([128, 512], F32, name="ps_o", tag="ps_w", bufs=2)
            for kk in range(2):
                nc.tensor.matmul(
                    po_t[:],
                    comb[kk][:, p * S + rc * 128 : p * S + (rc + 1) * 128],
                    wo_sb[:, kk * D + ob * 512 : kk * D + (ob + 1) * 512],
                    start=(kk == 0),
                    stop=(kk == 1),
                )
            if _op_n[0] % 2 == 0:
                nc.vector.tensor_copy(od[:, ob * 512 : (ob + 1) * 512], po_t[:])
            else:
                nc.scalar.copy(od[:, ob * 512 : (ob + 1) * 512], po_t[:])
            _op_n[0] += 1
        (nc.sync if rc % 2 == 0 else nc.gpsimd).dma_start(
            out=out_d[p * S + rc * 128 : p * S + (rc + 1) * 128, :], in_=od[:]
        )

    # ---- schedule -----------------------------------------------------
    # combos per p: L0..L3 then R0..R3; slot j = scores(j) + AV(j-1) with
    # AV interleaved at kc-pair grain; eager per-h combines; proj chunks and
    # the other p's output projection fill the PE between slots.
    combos = [(p, h, side) for p in range(2) for side in range(2) for h in range(NH)]

    # fillers keyed by combo index (run at slot start, before scores)
    fill = {
        1: [("proj", "v", 0), ("proj", "v", 1)],
        2: [("proj", "k", 2)],
        3: [("proj", "k", 3)],
        4: [("proj", "v", 2)],
        5: [("proj", "v", 3)],
        6: [("proj", "q", 2)],
        7: [("proj", "q", 3)],
        10: [("out", 0, 0), ("out", 0, 1)],
        11: [("out", 0, 2), ("out", 0, 3)],
        12: [("out", 0, 4)],
        13: [("out", 0, 5)],
        14: [("out", 0, 6)],
        15: [("out", 0, 7)],
    }

    def run_filler(f):
        if f[0] == "proj":
            proj_chunk(f[1], f[2])
        else:
            outproj_rc(f[1], f[2])

    # lead-in: k0,q0 -> L0 scores qb0 halves; k1 mid; q1 -> qb1 halves
    proj_chunk("k", 0)
    proj_chunk("q", 0)
    ex0 = new_ex()
    combo0 = combos[0]
    avs[combo0] = sb.tile([128, S], F32, name="av", tag="avT", bufs=7)
    for kc in range(KC):
        if kc == 4:
            proj_chunk("k", 1)
        ss = ps.tile([128, 1024], F32, name="ps_s", tag="ps_s", bufs=2)
        nc.tensor.matmul(
            ss[:, 0:512],
            kT[0][0:64, kc * 128 : (kc + 1) * 128],
            qT[0][0:64, 0:512],
            start=True,
            stop=True,
        )
        emit_exp(ex0, ss[:, 0:512], kc, qb=0)
    proj_chunk("q", 1)
    for kc in range(KC):
        ss = ps.tile([128, 1024], F32, name="ps_s", tag="ps_s", bufs=2)
        nc.tensor.matmul(
            ss[:, 0:512],
            kT[0][0:64, kc * 128 : (kc + 1) * 128],
            qT[0][0:64, 512:1024],
            start=True,
            stop=True,
        )
        emit_exp(ex0, ss[:, 0:512], kc, qb=1)

    prev = (combo0, ex0)
    pend_pre, pend_fin = [], []
    for j in range(1, len(combos) + 1):
        combo = combos[j] if j < len(combos) else None
        for f in fill.get(j, []):
            run_filler(f)
        if combo is not None:
            avs[combo] = sb.tile([128, S], F32, name="av", tag="avT", bufs=7)
            ex = new_ex()
        st = [None, None]
        for kc in range(KC):
            if combo is not None:
                score_kc(combo, ex, kc)
            if prev is not None and kc % 2 == 1:
                av_group(prev[0], prev[1], st, kc // 2)
        if prev is not None and prev[0][2] == 1:  # R side AV done
            pend_pre.append((prev[0][0], prev[0][1]))
        if pend_fin:
            combine_fin(*pend_fin.pop(0))
        if pend_pre:
            ph = pend_pre.pop(0)
            combine_pre(*ph)
            pend_fin.append(ph)
        prev = (combo, ex) if combo is not None else None

    # tail: last combine + p1 output projection
    while pend_fin:
        combine_fin(*pend_fin.pop(0))
    for rc in range(8):
        outproj_rc(1, rc)


_CACHED = None


def _build():
    global _CACHED
    if _CACHED is not None:
        return _CACHED
    nc = bacc.Bacc("TRN2", target_bir_lowering=False, debug=False)
    xq = nc.dram_tensor("xq", [D, R], X_DT, kind="ExternalInput")
    xk = nc.dram_tensor("xk", [D, R], X_DT, kind="ExternalInput")
    xv = nc.dram_tensor("xv", [D, R], X_DT, kind="ExternalInput")
    wq = nc.dram_tensor("wq", [D, CH], X_DT, kind="ExternalInput")
    wk = nc.dram_tensor("wk", [D, CH], X_DT, kind="ExternalInput")
    wv = nc.dram_tensor("wv", [D, CH], X_DT, kind="ExternalInput")
    wo = nc.dram_tensor("wo", [CH, D], OUT_DT, kind="ExternalInput")
    bq = nc.dram_tensor("bq", [CH], F32, kind="ExternalInput")
    bk = nc.dram_tensor("bk", [CH], F32, kind="ExternalInput")
    bv = nc.dram_tensor("bv", [CH], F32, kind="ExternalInput")
    mask = nc.dram_tensor("mask", [P, S], I32, kind="ExternalInput")
    out_d = nc.dram_tensor("out", [R, D], OUT_DT, kind="ExternalOutput")
    with tile.TileContext(nc) as tc:
        _emit(nc, tc, xq, xk, xv, wq, wk, wv, wo, bq, bk, bv, mask, out_d)
    nc.compile()
    _CACHED = nc
    return nc


def _in_maps(query, key, value, mask, Wq, bq, Wk, bk, Wv, bv, Wo):
    xnp = mybir.dt.np(X_DT)
    onp = mybir.dt.np(OUT_DT)
    f32 = lambda a: np.ascontiguousarray(np.asarray(a, dtype=np.float32))
    xdt = lambda a: np.ascontiguousarray(np.asarray(a).astype(xnp))
    odt = lambda a: np.ascontiguousarray(np.asarray(a).astype(onp))
    query, key, value = f32(query), f32(key), f32(value)
    Wq, Wk, Wv, Wo = f32(Wq), f32(Wk), f32(Wv), f32(Wo)
    bq, bk, bv = f32(bq), f32(bk), f32(bv)
    mask = np.ascontiguousarray(np.asarray(mask, dtype=np.int32))

    xqT = [xdt(query[b].reshape(R, D).T) for b in range(B)]
    xkT = [xdt(key[b].reshape(R, D).T) for b in range(B)]
    xvT = [xdt(value[b].reshape(R, D).T) for b in range(B)]

    maps = []
    for c in range(NCORES):
        b, hg = divmod(c, GROUP)
        ch = slice(hg * CH, (hg + 1) * CH)
        maps.append(
            {
                "xq": xqT[b],
                "xk": xkT[b],
                "xv": xvT[b],
                "wq": xdt(Wq[ch, :].T),
                "wk": xdt(Wk[ch, :].T),
                "wv": xdt(Wv[ch, :].T),
                "wo": odt(Wo[:, ch].T),
                "bq": bq[ch],
                "bk": bk[ch],
                "bv": bv[ch],
                "mask": mask[b, :, 0, :],
            }
        )
    return maps


def _run(in_maps, **kwargs):
    nc = _build()
    return run_bass_kernel_spmd(nc, in_maps, core_ids=list(range(NCORES)), **kwargs)


def kernel(query, key, value, mask, Wq, bq, Wk, bk, Wv, bv, Wo, bo):
    res = _run(_in_maps(query, key, value, mask, Wq, bq, Wk, bk, Wv, bv, Wo))
    bo = np.asarray(bo, dtype=np.float32)
    out = np.zeros((B, P, S, D), dtype=np.float32)
    for c in range(NCORES):
        b = c // GROUP
        out[b] += res.results[c]["out"].astype(np.float32).reshape(P, S, D)
    out += bo
    return out
